# revision 2
# baseline (speedup 1.0000x reference)
"""Multi-head attention Trainium2 kernel v3 (8 NeuronCores, SPMD).

Problem: B=4, T=2048, n_feat=512, H=8 heads, d_k=64.
Sharding: core c -> batch b = c//2, head-half hh = c%2 (4 heads, OB=256).

v3 design (bf16/f32r, accuracy-safe):
- fp8/DoubleRow abandoned: each fp8 quantization anywhere in the chain
  contributes its full ~4.4% relative error to the output (random-sum
  outputs do not average it away); measured 18% on the absmax metric.
- Inputs arrive bf16 (halves DMA), weights f32; all matmuls f32r/bf16 at
  1.0 cycles/row. Host still pre-transposes x and weights, folds bk/bv/bq
  biases (bk softmax-invariant, bv into the host output add, bq via the
  host exp-bias table ebias[j,h] = bq.k_j/8 + mask).
- No repack DMAs: Q/K live as [128, po, T] f32r with heads at partition
  halves; scores use 64-partition lhsT/rhs at base (h%2)*64.
- exp on ACT, 1024 wide, reading scores PSUM; softmax denominator from a
  ones-column in the PV matmul; norm via DVE reciprocal on the Z row,
  DMA lane-shift to partition 0, Pool partition_broadcast, DVE multiply.
- V projection rides ONE reused xp-slot tile with matmuls interleaved
  into the first attention pair (keeps the xp rotation parity; PE slack
  absorbs it).
- PSUM: st ping-pong 2x[128,1024] + xp 2x[128,1024] = 8 banks exactly.
"""
import sys

sys.path.insert(0, "/opt/trn_rl_repo")

import numpy as np

import concourse.bass as bass
import concourse.tile as tile
from concourse import bacc, mybir
from concourse.bass_utils import run_bass_kernel_spmd

P = 128
T = 2048
F = 512
OB = 256
NH = 4
DK = 64
JT = 16           # j tiles of 128
NSUP = 2
ISUP = T // NSUP  # 1024
FO = F // P       # 4
NEG = -1.0e30

f32 = mybir.dt.float32
f32r = mybir.dt.float32r
bf16 = mybir.dt.bfloat16
f16 = mybir.dt.float16
EXP = mybir.ActivationFunctionType.Exp

_CACHE = {}


def _build():
    nc = bacc.Bacc("TRN2", target_bir_lowering=False, debug=False, num_devices=8)

    xqT = nc.dram_tensor("xqT", (P, FO, T), f16, kind="ExternalInput").ap()
    xkT = nc.dram_tensor("xkT", (P, FO, T), f16, kind="ExternalInput").ap()
    xvT = nc.dram_tensor("xvT", (P, FO, T), f16, kind="ExternalInput").ap()
    wqT = nc.dram_tensor("wqT", (P, FO, OB), f16, kind="ExternalInput").ap()
    wkT = nc.dram_tensor("wkT", (P, FO, OB), f16, kind="ExternalInput").ap()
    wvT = nc.dram_tensor("wvT", (P, FO, OB), f16, kind="ExternalInput").ap()
    woT = nc.dram_tensor("woT", (P, 2, F), f32, kind="ExternalInput").ap()
    ebias = nc.dram_tensor("ebias", (P, NH, JT), f32, kind="ExternalInput").ap()
    outT = nc.dram_tensor("outT", (F, T), bf16, kind="ExternalOutput").ap()

    with tile.TileContext(nc) as tc:
        with tc.tile_pool(name="const", bufs=1) as cpool, \
             tc.tile_pool(name="persist", bufs=1) as ppool, \
             tc.tile_pool(name="et", bufs=5) as epool, \
             tc.tile_pool(name="norm", bufs=2) as npool, \
             tc.tile_pool(name="os", bufs=3) as opool, \
             tc.tile_pool(name="ps", bufs=1, space="PSUM") as ps:

            # manual PSUM slot control: four named 2-bank slots
            def slot(tag, name):
                return ps.tile([P, ISUP], f32, tag=tag, name=name)

            # ---- persistent SBUF tensors ----
            xq_sb = cpool.tile([P, FO, T], f16, tag="xq")
            xk_sb = cpool.tile([P, FO, T], f16, tag="xk")
            xv_sb = cpool.tile([P, FO, T], f16, tag="xv")
            wq_sb = cpool.tile([P, FO, OB], f16, tag="wq")
            wk_sb = cpool.tile([P, FO, OB], f16, tag="wk")
            wv_sb = cpool.tile([P, FO, OB], f16, tag="wv")
            wo_sb = cpool.tile([P, 2, F], f32r, tag="wo")
            wo_stg = cpool.tile([P, 2, F], f32, tag="wostg")
            eb_sb = cpool.tile([P, NH, JT], f32, tag="eb")

            # heads at partition halves: partitions (h%2)*64.. , dim1 = h//2
            QT = ppool.tile([P, 2, T], f32r, tag="QT")
            KT = ppool.tile([P, 2, T], f32r, tag="KT")
            V2 = ppool.tile([P, JT, NH, DK + 1], f32r, tag="V2")
            xT = ppool.tile([P, 2, T], f32r, tag="xT")
            xstg = ppool.tile([DK, 2, T], f32r, tag="xstg")

            # hoist the Exp act-table load to t~0
            ones64f = cpool.tile([DK + 1, DK], f32, tag="ones64f")
            nc.vector.memset(ones64f[:], 1.0)
            ones64 = cpool.tile([DK + 1, DK], f32r, tag="ones64")
            nc.vector.tensor_copy(ones64[:], ones64f[:])
            dumm = cpool.tile([1, 1], f32, tag="dumm")
            nc.vector.memset(dumm[:], 0.0)
            nc.scalar.activation(dumm[:], dumm[:], EXP)

            # ---- input loads (sync queue, in need order; xk in fo-halves
            # so the K projection starts accumulating early) ----
            nc.sync.dma_start(out=wk_sb[:], in_=wkT[:])
            nc.sync.dma_start(out=eb_sb[:], in_=ebias[:])
            nc.sync.dma_start(out=xk_sb[:, 0:2, :], in_=xkT[:, 0:2, :])
            nc.sync.dma_start(out=wq_sb[:], in_=wqT[:])
            nc.sync.dma_start(out=xk_sb[:, 2:4, :], in_=xkT[:, 2:4, :])
            nc.sync.dma_start(out=xq_sb[:, :, 0:ISUP], in_=xqT[:, :, 0:ISUP])
            nc.sync.dma_start(out=wv_sb[:], in_=wvT[:])

            def load_v_chunk(tg):
                nc.sync.dma_start(
                    out=xv_sb[:, :, tg * F:(tg + 1) * F],
                    in_=xvT[:, :, tg * F:(tg + 1) * F],
                )

            for tg in range(4):
                load_v_chunk(tg)
            nc.sync.dma_start(out=wo_stg[:], in_=woT[:])
            nc.vector.tensor_copy(wo_sb[:], wo_stg[:])
            nc.sync.dma_start(out=xq_sb[:, :, ISUP:T], in_=xqT[:, :, ISUP:T])

            # ones column of V2 (f32 memset + f32r-rounding copy; a bare
            # memset into f32r trips the BIR verifier)
            onesv = cpool.tile([P, JT * NH], f32, tag="onesv")
            nc.vector.memset(onesv[:], 1.0)
            nc.vector.tensor_copy(
                V2[:, :, :, DK:DK + 1],
                onesv[:].rearrange("p (j h) -> p j h ()", j=JT),
            )

            # ---- Q/K projection -> [128, po, T] f32r (no repack) ----
            def emit_qk_proj(w_sb, x_sb, dstT, po, tags, halves=(0, 1)):
                for half in halves:
                    pp = slot(tags[half % len(tags)], f"proj_{po}_{half}")
                    for fo in range(FO):
                        for c in range(2):
                            cl = slice(half * ISUP + c * 512,
                                       half * ISUP + (c + 1) * 512)
                            nc.tensor.matmul(
                                pp[:, c * 512:(c + 1) * 512],
                                w_sb[:, fo, po * P:(po + 1) * P],
                                x_sb[:, fo, cl],
                                start=(fo == 0),
                                stop=(fo == FO - 1),
                            )
                    nc.vector.tensor_copy(
                        dstT[:, po, half * ISUP:(half + 1) * ISUP], pp[:]
                    )

            def emit_v_proj(tg, tag):
                vstage = slot(tag, f"vstage_{tg}")
                vpr = vstage[:].rearrange("p (tt o) -> p tt o", tt=4)
                for tt in range(4):
                    t = 4 * tg + tt
                    for fo in range(FO):
                        nc.tensor.matmul(
                            vpr[:, tt, :],
                            xv_sb[:, fo, t * P:(t + 1) * P],
                            wv_sb[:, fo, :],
                            start=(fo == 0),
                            stop=(fo == FO - 1),
                        )
                nc.vector.tensor_copy(
                    V2[:, 4 * tg:4 * tg + 4, :, 0:DK],
                    vpr.rearrange("p tt (h d) -> p tt h d", h=NH),
                )

            # granular variants: yield once per ~2-matmul granule so the
            # jt loop can drip-feed PE and keep it continuously busy
            def gen_v_proj(tg, tag):
                vstage = slot(tag, f"vstage_{tg}")
                vpr = vstage[:].rearrange("p (tt o) -> p tt o", tt=4)
                for tt in range(4):
                    t = 4 * tg + tt
                    for fo in range(FO):
                        nc.tensor.matmul(
                            vpr[:, tt, :],
                            xv_sb[:, fo, t * P:(t + 1) * P],
                            wv_sb[:, fo, :],
                            start=(fo == 0),
                            stop=(fo == FO - 1),
                        )
                    yield
                nc.vector.tensor_copy(
                    V2[:, 4 * tg:4 * tg + 4, :, 0:DK],
                    vpr.rearrange("p tt (h d) -> p tt h d", h=NH),
                )
                yield

            def gen_qk_proj(w_sb, x_sb, dstT, po, tag, half):
                pp = slot(tag, f"gproj_{po}_{half}")
                for fo in range(FO):
                    for c in range(2):
                        cl = slice(half * ISUP + c * 512,
                                   half * ISUP + (c + 1) * 512)
                        nc.tensor.matmul(
                            pp[:, c * 512:(c + 1) * 512],
                            w_sb[:, fo, po * P:(po + 1) * P],
                            x_sb[:, fo, cl],
                            start=(fo == 0),
                            stop=(fo == FO - 1),
                        )
                    yield
                nc.vector.tensor_copy(
                    dstT[:, po, half * ISUP:(half + 1) * ISUP], pp[:]
                )
                yield

            # upfront: K po0 + Q po0 half0; ordering puts only the K-h0
            # and Q-h0 copies ahead of the first scores on DVE
            emit_qk_proj(wk_sb, xk_sb, KT, 0, ("st0",), halves=(0,))
            emit_qk_proj(wq_sb, xq_sb, QT, 0, ("st0",), halves=(0,))
            emit_qk_proj(wk_sb, xk_sb, KT, 0, ("st1",), halves=(1,))
            emit_v_proj(0, "xp1")

            # ---- norm: r = 1/Z (eps negligible: Z ~ 2e3), lane-shift to
            # partition 0, broadcast, scale ----
            def emit_norm(h, su, xp, c0=0, c1=1, w=ISUP):
                for c in range(c0, c1):
                    sl = slice(su * ISUP + c * w, su * ISUP + (c + 1) * w)
                    csl = slice(c * w, (c + 1) * w)
                    zt = npool.tile([P, ISUP], f32, tag="zt")
                    zrow = zt[DK:DK + 1, csl]
                    nc.vector.reciprocal(zrow, xp[DK:DK + 1, csl])
                    zr0 = npool.tile([1, ISUP], f32, tag="zr0")
                    nc.sync.dma_start(out=zr0[:, csl], in_=zrow)
                    rb = npool.tile([DK, ISUP], f32, tag="rb")
                    nc.gpsimd.partition_broadcast(rb[:, csl], zr0[:, csl])
                    if h < 2:
                        dst = xT[(h // 2) * DK:(h // 2) * DK + DK, h % 2, sl]
                        nc.vector.tensor_mul(dst, xp[0:DK, csl], rb[:, csl])
                    else:
                        dst = xstg[:, h % 2, sl]
                        nc.vector.tensor_mul(dst, xp[0:DK, csl], rb[:, csl])
                        nc.sync.dma_start(out=xT[DK:P, h % 2, sl], in_=dst)

            # ---- out-projection chunk: contraction od=256 as 2 matmuls ----
            def emit_outproj_chunk(su, op_tile, m, c, eng):
                sl = slice(su * ISUP + c * 512, su * ISUP + (c + 1) * 512)
                for r in range(2):
                    nc.tensor.matmul(
                        op_tile[:, 0:512],
                        wo_sb[:, r, m * P:(m + 1) * P],
                        xT[:, r, sl],
                        start=(r == 0),
                        stop=(r == 1),
                    )
                osb = opool.tile([P, 512], bf16, tag="osb")
                if eng == "act":
                    nc.scalar.copy(osb[:], op_tile[:, 0:512])
                else:
                    nc.vector.tensor_copy(osb[:], op_tile[:, 0:512])
                nc.sync.dma_start(out=outT[m * P:(m + 1) * P, sl], in_=osb[:])

            # ---- attention ----
            pending = [None]

            def dance_cb():
                if pending[0] is not None:
                    emit_norm(*pending[0])
                    pending[0] = None

            op_su0 = [None]
            # deferred PE work, drip-fed one granule per jt:
            #   idx0: V tg1-3 (xp1; tg0 is upfront), then K po1 h0 (xp1)
            #   idx1: K po1 h1, Q po1 h0/h1, Q po0 h1 (xp0, free after
            #         the idx0 dance)
            # consumers: idx2 needs K po1 + Q po1 h0; idx4 Q po1 h1;
            # idx6 Q po0 h1
            def deferred_work():
                for tg in (1, 2, 3):
                    yield from gen_v_proj(tg, "xp1")
                yield from gen_qk_proj(wk_sb, xk_sb, KT, 1, "xp1", 0)
                yield from gen_qk_proj(wk_sb, xk_sb, KT, 1, "xp0", 1)
                yield from gen_qk_proj(wq_sb, xq_sb, QT, 1, "xp0", 0)
                yield from gen_qk_proj(wq_sb, xq_sb, QT, 1, "xp0", 1)
                yield from gen_qk_proj(wq_sb, xq_sb, QT, 0, "xp0", 1)

            dwork = deferred_work()
            ddone = [False]

            def drip(n=1):
                if ddone[0]:
                    return
                try:
                    for _ in range(n):
                        next(dwork)
                except StopIteration:
                    ddone[0] = True

            # su0 runs po0 heads first (po1 projections are computed in
            # the background during idx0/idx1); su1 leads with heads 2/3
            # so the tail-critical last pair is head 0 (cheap norm)
            pairs = [(0, 1), (0, 0), (0, 2), (0, 3),
                     (1, 2), (1, 3), (1, 1), (1, 0)]
            for idx, (su, h) in enumerate(pairs):
                isl = su * ISUP
                hb = (h % 2) * DK
                po = h // 2
                xp = slot(f"xp{idx % 2}", f"xp_{su}_{h}")

                def scores(jt):
                    st = slot(f"st{jt % 2}", f"st_{su}_{h}_{jt}")
                    for c in range(2):
                        nc.tensor.matmul(
                            st[:, c * 512:(c + 1) * 512],
                            KT[hb:hb + DK, po, jt * P:(jt + 1) * P],
                            QT[hb:hb + DK, po,
                               isl + c * 512:isl + (c + 1) * 512],
                            start=True,
                            stop=True,
                        )
                    return st

                st_prev = scores(0)
                for jt in range(JT):
                    ett = epool.tile([P, ISUP], f32r, tag="et")
                    nc.scalar.activation(
                        ett[:], st_prev[:], EXP,
                        bias=eb_sb[:, h, jt:jt + 1], scale=0.125,
                    )
                    if jt + 1 < JT:
                        st_prev = scores(jt + 1)
                    if idx == 0:
                        # V tg1-3 must land before their PV jts: 2 granules
                        # per jt early on, then 1
                        drip(2 if jt < 8 else 1)
                    elif idx in (1, 2):
                        drip(1)
                    for c in range(2):
                        nc.tensor.matmul(
                            xp[0:DK + 1, c * 512:(c + 1) * 512],
                            V2[:, jt, h, :],
                            ett[:, c * 512:(c + 1) * 512],
                            start=(jt == 0),
                            stop=(jt == JT - 1),
                        )
                    if idx > 0 and jt == 2:
                        dance_cb()
                    if idx == NH:
                        # su0 fully normed; spread its out-projection on
                        # xp1 (idx3's slot, freed by the dance)
                        if jt == 4:
                            op_su0[0] = slot("xp1", "op_su0")
                        if 4 <= jt < 12 and jt % 2 == 0:
                            emit_outproj_chunk(
                                0, op_su0[0], (jt - 4) // 2, 0, "dve")
                        elif 4 <= jt < 12:
                            emit_outproj_chunk(
                                0, op_su0[0], (jt - 5) // 2, 1, "dve")
                pending[0] = (h, su, xp)

            # tail: last norm in halves; out-projection chunks over four
            # psum slots with ACT/DVE alternating copies
            (lh, lsu, lxp) = pending[0]
            pending[0] = None
            # tail norm, phase-grouped: recips -> PE-broadcast into a
            # PSUM slot (no DMA lane-shift / Pool launch on the critical
            # path) -> muls; then 8 chunks over a 3-slot rotation
            ops = [slot("xp0", "op_su1_a"), slot("st0", "op_su1_b"),
                   slot("xp1", "op_su1_c")]
            rb_ps = slot("st1", "op_su1_rb")
            zt = npool.tile([P, ISUP], f32r, tag="zt2")
            with nc.allow_low_precision(reason="1/Z fits f32r"):
                for c in range(2):
                    csl = slice(c * 512, (c + 1) * 512)
                    nc.vector.reciprocal(zt[DK:DK + 1, csl],
                                         lxp[DK:DK + 1, csl])
            for c in range(2):
                csl = slice(c * 512, (c + 1) * 512)
                nc.tensor.matmul(
                    rb_ps[0:DK, csl],
                    ones64[DK:DK + 1, :],
                    zt[DK:DK + 1, csl],
                    start=True, stop=True,
                )
            rb_sb = npool.tile([DK, ISUP], f32, tag="rbsb")
            for c in range(2):
                sl = slice(lsu * ISUP + c * 512, lsu * ISUP + (c + 1) * 512)
                csl = slice(c * 512, (c + 1) * 512)
                nc.vector.tensor_copy(rb_sb[:, csl], rb_ps[0:DK, csl])
                nc.vector.tensor_mul(
                    xT[(lh // 2) * DK:(lh // 2) * DK + DK, lh % 2, sl],
                    lxp[0:DK, csl],
                    rb_sb[:, csl],
                )
            for i, (m, c) in enumerate(
                [(0, 0), (1, 0), (2, 0), (3, 0),
                 (0, 1), (1, 1), (2, 1), (3, 1)]
            ):
                emit_outproj_chunk(1, ops[i % 3], m, c,
                                   "act" if i % 2 == 0 else "dve")

    nc.compile()
    return nc


def _prep_in_maps(query, key, value, mask, Wq, bq, Wk, bk, Wv, bv, Wo):
    f16np = np.float16
    B = query.shape[0]

    kfull = [key[b] @ Wk.T for b in range(B)]  # [T, 512] (no bk; see header)
    mrow = [np.where(mask[b, 0, :] == 0, np.float32(NEG), np.float32(0.0))
            for b in range(B)]

    def packT(x, npdt):
        # [T, F] -> [P, FO, T]
        return np.ascontiguousarray(
            x.T.reshape(FO, P, T).transpose(1, 0, 2)
        ).astype(npdt)

    def packW(w):
        # w [OB, F] -> [P, FO, OB] fp16
        return np.ascontiguousarray(
            w.T.reshape(FO, P, OB).transpose(1, 0, 2)
        ).astype(f16np)

    # woT[p, r, f] = Wo[f, ob0 + od(p, r)], od = ((p//64)*2 + r)*64 + p%64
    pp, rr = np.meshgrid(np.arange(P), np.arange(2), indexing="ij")
    od_map = ((pp // DK) * 2 + rr) * DK + (pp % DK)  # [P, 2]

    in_maps = []
    for c in range(8):
        b = c // 2
        hh = c % 2
        ob = slice(hh * OB, (hh + 1) * OB)
        kb = kfull[b][:, ob]
        bqb = bq[ob]
        eb = np.einsum(
            "jhd,hd->jh",
            kb.reshape(T, NH, DK),
            bqb.reshape(NH, DK),
        ) / 8.0
        eb = eb + mrow[b][:, None]
        eb = np.ascontiguousarray(
            eb.reshape(JT, P, NH).transpose(1, 2, 0)
        ).astype(np.float32)

        wo_blk = Wo[:, ob]
        woTv = np.ascontiguousarray(wo_blk.T[od_map, :]).astype(np.float32)

        in_maps.append({
            "xqT": packT(query[b], f16np),
            "xkT": packT(key[b], f16np),
            "xvT": packT(value[b], f16np),
            "wqT": packW(Wq[ob, :]),
            "wkT": packW(Wk[ob, :]),
            "wvT": packW(Wv[ob, :]),
            "woT": woTv,
            "ebias": eb,
        })
    return in_maps


def kernel(query, key, value, mask, Wq, bq, Wk, bk, Wv, bv, Wo, bo):
    query = np.asarray(query, dtype=np.float32)
    key = np.asarray(key, dtype=np.float32)
    value = np.asarray(value, dtype=np.float32)
    mask = np.asarray(mask)
    Wq = np.asarray(Wq, dtype=np.float32)
    bq = np.asarray(bq, dtype=np.float32)
    Wk = np.asarray(Wk, dtype=np.float32)
    bk = np.asarray(bk, dtype=np.float32)
    Wv = np.asarray(Wv, dtype=np.float32)
    bv = np.asarray(bv, dtype=np.float32)
    Wo = np.asarray(Wo, dtype=np.float32)
    bo = np.asarray(bo, dtype=np.float32)

    if "nc" not in _CACHE:
        _CACHE["nc"] = _build()
    nc = _CACHE["nc"]

    B = query.shape[0]
    in_maps = _prep_in_maps(
        query, key, value, mask, Wq, bq, Wk, bk, Wv, bv, Wo
    )
    res = run_bass_kernel_spmd(nc, in_maps, core_ids=list(range(8)))

    obias = bo + Wo @ bv
    out = np.empty((B, T, F), dtype=np.float32)
    for b in range(B):
        acc = (res.results[2 * b]["outT"].astype(np.float32)
               + res.results[2 * b + 1]["outT"].astype(np.float32))
        out[b] = acc.T + obias[None, :]
    return out


# revision 3
# speedup vs baseline: 1.0181x; 1.0181x over previous
"""Multi-head attention Trainium2 kernel v3 (8 NeuronCores, SPMD).

Problem: B=4, T=2048, n_feat=512, H=8 heads, d_k=64.
Sharding: core c -> batch b = c//2, head-half hh = c%2 (4 heads, OB=256).

v3 design (bf16/f32r, accuracy-safe):
- fp8/DoubleRow abandoned: each fp8 quantization anywhere in the chain
  contributes its full ~4.4% relative error to the output (random-sum
  outputs do not average it away); measured 18% on the absmax metric.
- Inputs arrive bf16 (halves DMA), weights f32; all matmuls f32r/bf16 at
  1.0 cycles/row. Host still pre-transposes x and weights, folds bk/bv/bq
  biases (bk softmax-invariant, bv into the host output add, bq via the
  host exp-bias table ebias[j,h] = bq.k_j/8 + mask).
- No repack DMAs: Q/K live as [128, po, T] f32r with heads at partition
  halves; scores use 64-partition lhsT/rhs at base (h%2)*64.
- exp on ACT, 1024 wide, reading scores PSUM; softmax denominator from a
  ones-column in the PV matmul; norm via DVE reciprocal on the Z row,
  DMA lane-shift to partition 0, Pool partition_broadcast, DVE multiply.
- V projection rides ONE reused xp-slot tile with matmuls interleaved
  into the first attention pair (keeps the xp rotation parity; PE slack
  absorbs it).
- PSUM: st ping-pong 2x[128,1024] + xp 2x[128,1024] = 8 banks exactly.
"""
import sys

sys.path.insert(0, "/opt/trn_rl_repo")

import numpy as np

import concourse.bass as bass
import concourse.tile as tile
from concourse import bacc, mybir
from concourse.bass_utils import run_bass_kernel_spmd

P = 128
T = 2048
F = 512
OB = 256
NH = 4
DK = 64
JT = 16           # j tiles of 128
NSUP = 2
ISUP = T // NSUP  # 1024
FO = F // P       # 4
NEG = -1.0e30

f32 = mybir.dt.float32
f32r = mybir.dt.float32r
bf16 = mybir.dt.bfloat16
f16 = mybir.dt.float16
EXP = mybir.ActivationFunctionType.Exp

_CACHE = {}


def _build():
    nc = bacc.Bacc("TRN2", target_bir_lowering=False, debug=False, num_devices=8)

    xqT = nc.dram_tensor("xqT", (P, FO, T), f16, kind="ExternalInput").ap()
    xkT = nc.dram_tensor("xkT", (P, FO, T), f16, kind="ExternalInput").ap()
    xvT = nc.dram_tensor("xvT", (P, FO, T), f16, kind="ExternalInput").ap()
    wqT = nc.dram_tensor("wqT", (P, FO, OB), f16, kind="ExternalInput").ap()
    wkT = nc.dram_tensor("wkT", (P, FO, OB), f16, kind="ExternalInput").ap()
    wvT = nc.dram_tensor("wvT", (P, FO, OB), f16, kind="ExternalInput").ap()
    woT = nc.dram_tensor("woT", (P, 2, F), f32, kind="ExternalInput").ap()
    ebias = nc.dram_tensor("ebias", (P, NH, JT), f32, kind="ExternalInput").ap()
    outT = nc.dram_tensor("outT", (F, T), bf16, kind="ExternalOutput").ap()

    with tile.TileContext(nc) as tc:
        with tc.tile_pool(name="const", bufs=1) as cpool, \
             tc.tile_pool(name="persist", bufs=1) as ppool, \
             tc.tile_pool(name="et", bufs=5) as epool, \
             tc.tile_pool(name="norm", bufs=2) as npool, \
             tc.tile_pool(name="os", bufs=4) as opool, \
             tc.tile_pool(name="ps", bufs=1, space="PSUM") as ps:

            # manual PSUM slot control: four named 2-bank slots
            def slot(tag, name):
                return ps.tile([P, ISUP], f32, tag=tag, name=name)

            # ---- persistent SBUF tensors ----
            xq_sb = cpool.tile([P, FO, T], f16, tag="xq")
            xk_sb = cpool.tile([P, FO, T], f16, tag="xk")
            xv_sb = cpool.tile([P, FO, T], f16, tag="xv")
            wq_sb = cpool.tile([P, FO, OB], f16, tag="wq")
            wk_sb = cpool.tile([P, FO, OB], f16, tag="wk")
            wv_sb = cpool.tile([P, FO, OB], f16, tag="wv")
            wo_sb = cpool.tile([P, 2, F], f32r, tag="wo")
            wo_stg = cpool.tile([P, 2, F], f32, tag="wostg")
            eb_sb = cpool.tile([P, NH, JT], f32, tag="eb")

            # heads at partition halves: partitions (h%2)*64.. , dim1 = h//2
            QT = ppool.tile([P, 2, T], f32r, tag="QT")
            KT = ppool.tile([P, 2, T], f32r, tag="KT")
            V2 = ppool.tile([P, JT, NH, DK + 1], f32r, tag="V2")
            xT = ppool.tile([P, 2, T], f32r, tag="xT")
            xstg = ppool.tile([DK, 2, T], f32r, tag="xstg")

            # hoist the Exp act-table load to t~0
            ones64f = cpool.tile([DK + 1, DK], f32, tag="ones64f")
            nc.vector.memset(ones64f[:], 1.0)
            ones64 = cpool.tile([DK + 1, DK], f32r, tag="ones64")
            nc.vector.tensor_copy(ones64[:], ones64f[:])
            dumm = cpool.tile([1, 1], f32, tag="dumm")
            nc.vector.memset(dumm[:], 0.0)
            nc.scalar.activation(dumm[:], dumm[:], EXP)

            # ---- input loads (sync queue, in need order; xk in fo-halves
            # so the K projection starts accumulating early) ----
            nc.sync.dma_start(out=wk_sb[:], in_=wkT[:])
            nc.sync.dma_start(out=eb_sb[:], in_=ebias[:])
            nc.sync.dma_start(out=xk_sb[:, 0:2, :], in_=xkT[:, 0:2, :])
            nc.sync.dma_start(out=wq_sb[:], in_=wqT[:])
            nc.sync.dma_start(out=xk_sb[:, 2:4, :], in_=xkT[:, 2:4, :])
            nc.sync.dma_start(out=xq_sb[:, :, 0:ISUP], in_=xqT[:, :, 0:ISUP])
            nc.sync.dma_start(out=wv_sb[:], in_=wvT[:])

            def load_v_chunk(tg):
                nc.sync.dma_start(
                    out=xv_sb[:, :, tg * F:(tg + 1) * F],
                    in_=xvT[:, :, tg * F:(tg + 1) * F],
                )

            for tg in range(4):
                load_v_chunk(tg)
            nc.sync.dma_start(out=wo_stg[:], in_=woT[:])
            nc.vector.tensor_copy(wo_sb[:], wo_stg[:])
            nc.sync.dma_start(out=xq_sb[:, :, ISUP:T], in_=xqT[:, :, ISUP:T])

            # ones column of V2 (f32 memset + f32r-rounding copy; a bare
            # memset into f32r trips the BIR verifier)
            onesv = cpool.tile([P, JT * NH], f32, tag="onesv")
            nc.vector.memset(onesv[:], 1.0)
            nc.vector.tensor_copy(
                V2[:, :, :, DK:DK + 1],
                onesv[:].rearrange("p (j h) -> p j h ()", j=JT),
            )

            # ---- Q/K projection -> [128, po, T] f32r (no repack) ----
            def emit_qk_proj(w_sb, x_sb, dstT, po, tags, halves=(0, 1)):
                for half in halves:
                    pp = slot(tags[half % len(tags)], f"proj_{po}_{half}")
                    for fo in range(FO):
                        for c in range(2):
                            cl = slice(half * ISUP + c * 512,
                                       half * ISUP + (c + 1) * 512)
                            nc.tensor.matmul(
                                pp[:, c * 512:(c + 1) * 512],
                                w_sb[:, fo, po * P:(po + 1) * P],
                                x_sb[:, fo, cl],
                                start=(fo == 0),
                                stop=(fo == FO - 1),
                            )
                    nc.vector.tensor_copy(
                        dstT[:, po, half * ISUP:(half + 1) * ISUP], pp[:]
                    )

            def emit_v_proj(tg, tag):
                vstage = slot(tag, f"vstage_{tg}")
                vpr = vstage[:].rearrange("p (tt o) -> p tt o", tt=4)
                for tt in range(4):
                    t = 4 * tg + tt
                    for fo in range(FO):
                        nc.tensor.matmul(
                            vpr[:, tt, :],
                            xv_sb[:, fo, t * P:(t + 1) * P],
                            wv_sb[:, fo, :],
                            start=(fo == 0),
                            stop=(fo == FO - 1),
                        )
                nc.vector.tensor_copy(
                    V2[:, 4 * tg:4 * tg + 4, :, 0:DK],
                    vpr.rearrange("p tt (h d) -> p tt h d", h=NH),
                )

            # granular variants: yield once per ~2-matmul granule so the
            # jt loop can drip-feed PE and keep it continuously busy
            def gen_v_proj(tg, tag):
                vstage = slot(tag, f"vstage_{tg}")
                vpr = vstage[:].rearrange("p (tt o) -> p tt o", tt=4)
                for tt in range(4):
                    t = 4 * tg + tt
                    for fo in range(FO):
                        nc.tensor.matmul(
                            vpr[:, tt, :],
                            xv_sb[:, fo, t * P:(t + 1) * P],
                            wv_sb[:, fo, :],
                            start=(fo == 0),
                            stop=(fo == FO - 1),
                        )
                    yield
                nc.vector.tensor_copy(
                    V2[:, 4 * tg:4 * tg + 4, :, 0:DK],
                    vpr.rearrange("p tt (h d) -> p tt h d", h=NH),
                )
                yield

            def gen_qk_proj(w_sb, x_sb, dstT, po, tag, half):
                pp = slot(tag, f"gproj_{po}_{half}")
                for fo in range(FO):
                    for c in range(2):
                        cl = slice(half * ISUP + c * 512,
                                   half * ISUP + (c + 1) * 512)
                        nc.tensor.matmul(
                            pp[:, c * 512:(c + 1) * 512],
                            w_sb[:, fo, po * P:(po + 1) * P],
                            x_sb[:, fo, cl],
                            start=(fo == 0),
                            stop=(fo == FO - 1),
                        )
                    yield
                nc.vector.tensor_copy(
                    dstT[:, po, half * ISUP:(half + 1) * ISUP], pp[:]
                )
                yield

            # PE warm-up: scratch matmuls bridge the input-DMA wait so the
            # p-state ramp reaches full clock before the real projections
            # (cold PE runs them at 2-4x the cycle time)
            warm = slot("st1", "warmup")
            for _ in range(60):
                nc.tensor.matmul(
                    warm[0:1, 0:DK],
                    ones64[DK:DK + 1, 0:1],
                    ones64[DK:DK + 1, :],
                    start=True, stop=True,
                )

            # upfront: K po0 + Q po0 half0; ordering puts only the K-h0
            # and Q-h0 copies ahead of the first scores on DVE
            emit_qk_proj(wk_sb, xk_sb, KT, 0, ("st0",), halves=(0,))
            emit_qk_proj(wq_sb, xq_sb, QT, 0, ("st0",), halves=(0,))
            emit_qk_proj(wk_sb, xk_sb, KT, 0, ("st1",), halves=(1,))
            emit_v_proj(0, "xp1")

            # ---- norm: r = 1/Z (eps negligible: Z ~ 2e3), lane-shift to
            # partition 0, broadcast, scale ----
            def emit_norm(h, su, xp, c0=0, c1=1, w=ISUP):
                for c in range(c0, c1):
                    sl = slice(su * ISUP + c * w, su * ISUP + (c + 1) * w)
                    csl = slice(c * w, (c + 1) * w)
                    zt = npool.tile([P, ISUP], f32, tag="zt")
                    zrow = zt[DK:DK + 1, csl]
                    nc.vector.reciprocal(zrow, xp[DK:DK + 1, csl])
                    zr0 = npool.tile([1, ISUP], f32, tag="zr0")
                    nc.sync.dma_start(out=zr0[:, csl], in_=zrow)
                    rb = npool.tile([DK, ISUP], f32, tag="rb")
                    nc.gpsimd.partition_broadcast(rb[:, csl], zr0[:, csl])
                    if h < 2:
                        dst = xT[(h // 2) * DK:(h // 2) * DK + DK, h % 2, sl]
                        nc.vector.tensor_mul(dst, xp[0:DK, csl], rb[:, csl])
                    else:
                        dst = xstg[:, h % 2, sl]
                        nc.vector.tensor_mul(dst, xp[0:DK, csl], rb[:, csl])
                        nc.sync.dma_start(out=xT[DK:P, h % 2, sl], in_=dst)

            # ---- out-projection chunk: contraction od=256 as 2 matmuls ----
            def emit_outproj_chunk(su, op_tile, m, c, eng):
                sl = slice(su * ISUP + c * 512, su * ISUP + (c + 1) * 512)
                for r in range(2):
                    nc.tensor.matmul(
                        op_tile[:, 0:512],
                        wo_sb[:, r, m * P:(m + 1) * P],
                        xT[:, r, sl],
                        start=(r == 0),
                        stop=(r == 1),
                    )
                osb = opool.tile([P, 512], bf16, tag="osb")
                if eng == "act":
                    nc.scalar.copy(osb[:], op_tile[:, 0:512])
                    nc.scalar.dma_start(
                        out=outT[m * P:(m + 1) * P, sl], in_=osb[:])
                else:
                    nc.vector.tensor_copy(osb[:], op_tile[:, 0:512])
                    nc.sync.dma_start(
                        out=outT[m * P:(m + 1) * P, sl], in_=osb[:])

            # ---- attention ----
            pending = [None]

            def dance_cb():
                if pending[0] is not None:
                    emit_norm(*pending[0])
                    pending[0] = None

            op_su0 = [None]
            # deferred PE work, drip-fed one granule per jt:
            #   idx0: V tg1-3 (xp1; tg0 is upfront), then K po1 h0 (xp1)
            #   idx1: K po1 h1, Q po1 h0/h1, Q po0 h1 (xp0, free after
            #         the idx0 dance)
            # consumers: idx2 needs K po1 + Q po1 h0; idx4 Q po1 h1;
            # idx6 Q po0 h1
            def deferred_work():
                for tg in (1, 2, 3):
                    yield from gen_v_proj(tg, "xp1")
                yield from gen_qk_proj(wk_sb, xk_sb, KT, 1, "xp1", 0)
                yield from gen_qk_proj(wk_sb, xk_sb, KT, 1, "xp0", 1)
                yield from gen_qk_proj(wq_sb, xq_sb, QT, 1, "xp0", 0)
                yield from gen_qk_proj(wq_sb, xq_sb, QT, 1, "xp0", 1)
                yield from gen_qk_proj(wq_sb, xq_sb, QT, 0, "xp0", 1)

            dwork = deferred_work()
            ddone = [False]

            def drip(n=1):
                if ddone[0]:
                    return
                try:
                    for _ in range(n):
                        next(dwork)
                except StopIteration:
                    ddone[0] = True

            # su0 runs po0 heads first (po1 projections are computed in
            # the background during idx0/idx1); su1 leads with heads 2/3
            # so the tail-critical last pair is head 0 (cheap norm)
            pairs = [(0, 1), (0, 0), (0, 2), (0, 3),
                     (1, 2), (1, 3), (1, 1), (1, 0)]
            for idx, (su, h) in enumerate(pairs):
                isl = su * ISUP
                hb = (h % 2) * DK
                po = h // 2
                xp = slot(f"xp{idx % 2}", f"xp_{su}_{h}")

                def scores(jt):
                    st = slot(f"st{jt % 2}", f"st_{su}_{h}_{jt}")
                    for c in range(2):
                        nc.tensor.matmul(
                            st[:, c * 512:(c + 1) * 512],
                            KT[hb:hb + DK, po, jt * P:(jt + 1) * P],
                            QT[hb:hb + DK, po,
                               isl + c * 512:isl + (c + 1) * 512],
                            start=True,
                            stop=True,
                        )
                    return st

                st_prev = scores(0)
                for jt in range(JT):
                    ett = epool.tile([P, ISUP], f32r, tag="et")
                    nc.scalar.activation(
                        ett[:], st_prev[:], EXP,
                        bias=eb_sb[:, h, jt:jt + 1], scale=0.125,
                    )
                    if jt + 1 < JT:
                        st_prev = scores(jt + 1)
                    if idx == 0:
                        # V deadlines force ~2 granules/jt early on
                        drip(2 if jt < 8 else 1)
                    elif idx in (1, 2, 3) and jt % 2 == 1:
                        # half-rate drip fits inside PE's per-jt slack
                        drip(1)
                    for c in range(2):
                        nc.tensor.matmul(
                            xp[0:DK + 1, c * 512:(c + 1) * 512],
                            V2[:, jt, h, :],
                            ett[:, c * 512:(c + 1) * 512],
                            start=(jt == 0),
                            stop=(jt == JT - 1),
                        )
                    if idx > 0 and jt == 2:
                        dance_cb()
                    if idx == NH:
                        # su0 fully normed; spread its out-projection on
                        # xp1 (idx3's slot, freed by the dance)
                        if jt == 4:
                            op_su0[0] = slot("xp1", "op_su0")
                        if 4 <= jt < 12 and jt % 2 == 0:
                            emit_outproj_chunk(
                                0, op_su0[0], (jt - 4) // 2, 0, "dve")
                        elif 4 <= jt < 12:
                            emit_outproj_chunk(
                                0, op_su0[0], (jt - 5) // 2, 1, "dve")
                pending[0] = (h, su, xp)

            # tail: last norm in halves; out-projection chunks over four
            # psum slots with ACT/DVE alternating copies
            (lh, lsu, lxp) = pending[0]
            pending[0] = None
            # tail norm, phase-grouped: recips -> PE-broadcast into a
            # PSUM slot (no DMA lane-shift / Pool launch on the critical
            # path) -> muls; then 8 chunks over a 3-slot rotation
            ops = [slot("xp0", "op_su1_a"), slot("st0", "op_su1_b"),
                   slot("xp1", "op_su1_c")]
            rb_ps = slot("st1", "op_su1_rb")
            zt = npool.tile([P, ISUP], f32r, tag="zt2")
            with nc.allow_low_precision(reason="1/Z fits f32r"):
                nc.vector.reciprocal(zt[DK:DK + 1, :], lxp[DK:DK + 1, :])
            for c in range(2):
                csl = slice(c * 512, (c + 1) * 512)
                nc.tensor.matmul(
                    rb_ps[0:DK, csl],
                    ones64[DK:DK + 1, :],
                    zt[DK:DK + 1, csl],
                    start=True, stop=True,
                )
            rb_sb = npool.tile([DK, ISUP], f32, tag="rbsb")
            nc.vector.tensor_copy(rb_sb[:], rb_ps[0:DK, :])
            for c in range(2):
                sl = slice(lsu * ISUP + c * 512, lsu * ISUP + (c + 1) * 512)
                csl = slice(c * 512, (c + 1) * 512)
                nc.vector.tensor_mul(
                    xT[(lh // 2) * DK:(lh // 2) * DK + DK, lh % 2, sl],
                    lxp[0:DK, csl],
                    rb_sb[:, csl],
                )
            for i, (m, c) in enumerate(
                [(0, 0), (1, 0), (2, 0), (3, 0),
                 (0, 1), (1, 1), (2, 1), (3, 1)]
            ):
                emit_outproj_chunk(1, ops[i % 3], m, c,
                                   "dve" if i in (2, 5) else "act")

    nc.compile()
    return nc


def _prep_in_maps(query, key, value, mask, Wq, bq, Wk, bk, Wv, bv, Wo):
    f16np = np.float16
    B = query.shape[0]

    kfull = [key[b] @ Wk.T for b in range(B)]  # [T, 512] (no bk; see header)
    mrow = [np.where(mask[b, 0, :] == 0, np.float32(NEG), np.float32(0.0))
            for b in range(B)]

    def packT(x, npdt):
        # [T, F] -> [P, FO, T]
        return np.ascontiguousarray(
            x.T.reshape(FO, P, T).transpose(1, 0, 2)
        ).astype(npdt)

    def packW(w):
        # w [OB, F] -> [P, FO, OB] fp16
        return np.ascontiguousarray(
            w.T.reshape(FO, P, OB).transpose(1, 0, 2)
        ).astype(f16np)

    # woT[p, r, f] = Wo[f, ob0 + od(p, r)], od = ((p//64)*2 + r)*64 + p%64
    pp, rr = np.meshgrid(np.arange(P), np.arange(2), indexing="ij")
    od_map = ((pp // DK) * 2 + rr) * DK + (pp % DK)  # [P, 2]

    in_maps = []
    for c in range(8):
        b = c // 2
        hh = c % 2
        ob = slice(hh * OB, (hh + 1) * OB)
        kb = kfull[b][:, ob]
        bqb = bq[ob]
        eb = np.einsum(
            "jhd,hd->jh",
            kb.reshape(T, NH, DK),
            bqb.reshape(NH, DK),
        ) / 8.0
        eb = eb + mrow[b][:, None]
        eb = np.ascontiguousarray(
            eb.reshape(JT, P, NH).transpose(1, 2, 0)
        ).astype(np.float32)

        wo_blk = Wo[:, ob]
        woTv = np.ascontiguousarray(wo_blk.T[od_map, :]).astype(np.float32)

        in_maps.append({
            "xqT": packT(query[b], f16np),
            "xkT": packT(key[b], f16np),
            "xvT": packT(value[b], f16np),
            "wqT": packW(Wq[ob, :]),
            "wkT": packW(Wk[ob, :]),
            "wvT": packW(Wv[ob, :]),
            "woT": woTv,
            "ebias": eb,
        })
    return in_maps


def kernel(query, key, value, mask, Wq, bq, Wk, bk, Wv, bv, Wo, bo):
    query = np.asarray(query, dtype=np.float32)
    key = np.asarray(key, dtype=np.float32)
    value = np.asarray(value, dtype=np.float32)
    mask = np.asarray(mask)
    Wq = np.asarray(Wq, dtype=np.float32)
    bq = np.asarray(bq, dtype=np.float32)
    Wk = np.asarray(Wk, dtype=np.float32)
    bk = np.asarray(bk, dtype=np.float32)
    Wv = np.asarray(Wv, dtype=np.float32)
    bv = np.asarray(bv, dtype=np.float32)
    Wo = np.asarray(Wo, dtype=np.float32)
    bo = np.asarray(bo, dtype=np.float32)

    if "nc" not in _CACHE:
        _CACHE["nc"] = _build()
    nc = _CACHE["nc"]

    B = query.shape[0]
    in_maps = _prep_in_maps(
        query, key, value, mask, Wq, bq, Wk, bk, Wv, bv, Wo
    )
    res = run_bass_kernel_spmd(nc, in_maps, core_ids=list(range(8)))

    obias = bo + Wo @ bv
    out = np.empty((B, T, F), dtype=np.float32)
    for b in range(B):
        acc = (res.results[2 * b]["outT"].astype(np.float32)
               + res.results[2 * b + 1]["outT"].astype(np.float32))
        out[b] = acc.T + obias[None, :]
    return out


# revision 4
# speedup vs baseline: 1.0191x; 1.0009x over previous
"""Multi-head attention Trainium2 kernel v3 (8 NeuronCores, SPMD).

Problem: B=4, T=2048, n_feat=512, H=8 heads, d_k=64.
Sharding: core c -> batch b = c//2, head-half hh = c%2 (4 heads, OB=256).

v3 design (bf16/f32r, accuracy-safe):
- fp8/DoubleRow abandoned: each fp8 quantization anywhere in the chain
  contributes its full ~4.4% relative error to the output (random-sum
  outputs do not average it away); measured 18% on the absmax metric.
- Inputs arrive bf16 (halves DMA), weights f32; all matmuls f32r/bf16 at
  1.0 cycles/row. Host still pre-transposes x and weights, folds bk/bv/bq
  biases (bk softmax-invariant, bv into the host output add, bq via the
  host exp-bias table ebias[j,h] = bq.k_j/8 + mask).
- No repack DMAs: Q/K live as [128, po, T] f32r with heads at partition
  halves; scores use 64-partition lhsT/rhs at base (h%2)*64.
- exp on ACT, 1024 wide, reading scores PSUM; softmax denominator from a
  ones-column in the PV matmul; norm via DVE reciprocal on the Z row,
  DMA lane-shift to partition 0, Pool partition_broadcast, DVE multiply.
- V projection rides ONE reused xp-slot tile with matmuls interleaved
  into the first attention pair (keeps the xp rotation parity; PE slack
  absorbs it).
- PSUM: st ping-pong 2x[128,1024] + xp 2x[128,1024] = 8 banks exactly.
"""
import sys

sys.path.insert(0, "/opt/trn_rl_repo")

import numpy as np

import concourse.bass as bass
import concourse.tile as tile
from concourse import bacc, mybir
from concourse.bass_utils import run_bass_kernel_spmd

P = 128
T = 2048
F = 512
OB = 256
NH = 4
DK = 64
JT = 16           # j tiles of 128
NSUP = 2
ISUP = T // NSUP  # 1024
FO = F // P       # 4
NEG = -1.0e30

f32 = mybir.dt.float32
f32r = mybir.dt.float32r
bf16 = mybir.dt.bfloat16
f16 = mybir.dt.float16
EXP = mybir.ActivationFunctionType.Exp

_CACHE = {}


def _build():
    nc = bacc.Bacc("TRN2", target_bir_lowering=False, debug=False, num_devices=8)

    xqT = nc.dram_tensor("xqT", (P, FO, T), f16, kind="ExternalInput").ap()
    xkT = nc.dram_tensor("xkT", (P, FO, T), f16, kind="ExternalInput").ap()
    xvT = nc.dram_tensor("xvT", (P, FO, T), f16, kind="ExternalInput").ap()
    wqT = nc.dram_tensor("wqT", (P, FO, OB), f16, kind="ExternalInput").ap()
    wkT = nc.dram_tensor("wkT", (P, FO, OB), f16, kind="ExternalInput").ap()
    wvT = nc.dram_tensor("wvT", (P, FO, OB), f16, kind="ExternalInput").ap()
    woT = nc.dram_tensor("woT", (P, 2, F), f32, kind="ExternalInput").ap()
    ebias = nc.dram_tensor("ebias", (P, NH, JT), f32, kind="ExternalInput").ap()
    outT = nc.dram_tensor("outT", (F, T), bf16, kind="ExternalOutput").ap()

    with tile.TileContext(nc) as tc:
        with tc.tile_pool(name="const", bufs=1) as cpool, \
             tc.tile_pool(name="persist", bufs=1) as ppool, \
             tc.tile_pool(name="et", bufs=5) as epool, \
             tc.tile_pool(name="norm", bufs=2) as npool, \
             tc.tile_pool(name="os", bufs=4) as opool, \
             tc.tile_pool(name="ps", bufs=1, space="PSUM") as ps:

            # manual PSUM slot control: four named 2-bank slots
            def slot(tag, name):
                return ps.tile([P, ISUP], f32, tag=tag, name=name)

            # ---- persistent SBUF tensors ----
            xq_sb = cpool.tile([P, FO, T], f16, tag="xq")
            xk_sb = cpool.tile([P, FO, T], f16, tag="xk")
            xv_sb = cpool.tile([P, FO, T], f16, tag="xv")
            wq_sb = cpool.tile([P, FO, OB], f16, tag="wq")
            wk_sb = cpool.tile([P, FO, OB], f16, tag="wk")
            wv_sb = cpool.tile([P, FO, OB], f16, tag="wv")
            wo_sb = cpool.tile([P, 2, F], f32r, tag="wo")
            wo_stg = cpool.tile([P, 2, F], f32, tag="wostg")
            eb_sb = cpool.tile([P, NH, JT], f32, tag="eb")

            # heads at partition halves: partitions (h%2)*64.. , dim1 = h//2
            QT = ppool.tile([P, 2, T], f32r, tag="QT")
            KT = ppool.tile([P, 2, T], f32r, tag="KT")
            V2 = ppool.tile([P, JT, NH, DK + 1], f32r, tag="V2")
            xT = ppool.tile([P, 2, T], f32r, tag="xT")
            xstg = ppool.tile([DK, 2, T], f32r, tag="xstg")

            # hoist the Exp act-table load to t~0
            ones64f = cpool.tile([DK + 1, DK], f32, tag="ones64f")
            nc.vector.memset(ones64f[:], 1.0)
            ones64 = cpool.tile([DK + 1, DK], f32r, tag="ones64")
            nc.vector.tensor_copy(ones64[:], ones64f[:])
            dumm = cpool.tile([1, 1], f32, tag="dumm")
            nc.vector.memset(dumm[:], 0.0)
            nc.scalar.activation(dumm[:], dumm[:], EXP)

            # ---- input loads (sync queue, in need order; xk in fo-halves
            # so the K projection starts accumulating early) ----
            nc.sync.dma_start(out=wk_sb[:], in_=wkT[:])
            nc.sync.dma_start(out=eb_sb[:], in_=ebias[:])
            nc.sync.dma_start(out=xk_sb[:, 0:2, :], in_=xkT[:, 0:2, :])
            nc.sync.dma_start(out=wq_sb[:], in_=wqT[:])
            nc.sync.dma_start(out=xk_sb[:, 2:4, :], in_=xkT[:, 2:4, :])
            nc.sync.dma_start(out=xq_sb[:, :, 0:ISUP], in_=xqT[:, :, 0:ISUP])
            nc.sync.dma_start(out=wv_sb[:], in_=wvT[:])

            def load_v_chunk(tg):
                nc.sync.dma_start(
                    out=xv_sb[:, :, tg * F:(tg + 1) * F],
                    in_=xvT[:, :, tg * F:(tg + 1) * F],
                )

            for tg in range(4):
                load_v_chunk(tg)
            nc.sync.dma_start(out=wo_stg[:], in_=woT[:])
            nc.vector.tensor_copy(wo_sb[:], wo_stg[:])
            nc.sync.dma_start(out=xq_sb[:, :, ISUP:T], in_=xqT[:, :, ISUP:T])

            # ones column of V2 (f32 memset + f32r-rounding copy; a bare
            # memset into f32r trips the BIR verifier)
            onesv = cpool.tile([P, JT * NH], f32, tag="onesv")
            nc.vector.memset(onesv[:], 1.0)
            nc.vector.tensor_copy(
                V2[:, :, :, DK:DK + 1],
                onesv[:].rearrange("p (j h) -> p j h ()", j=JT),
            )

            # ---- Q/K projection -> [128, po, T] f32r (no repack) ----
            def emit_qk_proj(w_sb, x_sb, dstT, po, tags, halves=(0, 1),
                             split_copy=False):
                for half in halves:
                    pp = slot(tags[half % len(tags)], f"proj_{po}_{half}")
                    for fo in range(FO):
                        for c in range(2):
                            cl = slice(half * ISUP + c * 512,
                                       half * ISUP + (c + 1) * 512)
                            nc.tensor.matmul(
                                pp[:, c * 512:(c + 1) * 512],
                                w_sb[:, fo, po * P:(po + 1) * P],
                                x_sb[:, fo, cl],
                                start=(fo == 0),
                                stop=(fo == FO - 1),
                            )
                    dst = dstT[:, po, half * ISUP:(half + 1) * ISUP]
                    if split_copy:
                        nc.vector.tensor_copy(dst[:, 0:512], pp[:, 0:512])
                        nc.scalar.copy(dst[:, 512:ISUP], pp[:, 512:ISUP])
                    else:
                        nc.vector.tensor_copy(dst, pp[:])

            def emit_v_proj(tg, tag):
                vstage = slot(tag, f"vstage_{tg}")
                vpr = vstage[:].rearrange("p (tt o) -> p tt o", tt=4)
                for tt in range(4):
                    t = 4 * tg + tt
                    for fo in range(FO):
                        nc.tensor.matmul(
                            vpr[:, tt, :],
                            xv_sb[:, fo, t * P:(t + 1) * P],
                            wv_sb[:, fo, :],
                            start=(fo == 0),
                            stop=(fo == FO - 1),
                        )
                nc.vector.tensor_copy(
                    V2[:, 4 * tg:4 * tg + 4, :, 0:DK],
                    vpr.rearrange("p tt (h d) -> p tt h d", h=NH),
                )

            # granular variants: yield once per ~2-matmul granule so the
            # jt loop can drip-feed PE and keep it continuously busy
            def gen_v_proj(tg, tag):
                vstage = slot(tag, f"vstage_{tg}")
                vpr = vstage[:].rearrange("p (tt o) -> p tt o", tt=4)
                for tt in range(4):
                    t = 4 * tg + tt
                    for fo in range(FO):
                        nc.tensor.matmul(
                            vpr[:, tt, :],
                            xv_sb[:, fo, t * P:(t + 1) * P],
                            wv_sb[:, fo, :],
                            start=(fo == 0),
                            stop=(fo == FO - 1),
                        )
                    yield
                nc.vector.tensor_copy(
                    V2[:, 4 * tg:4 * tg + 4, :, 0:DK],
                    vpr.rearrange("p tt (h d) -> p tt h d", h=NH),
                )
                yield

            def gen_qk_proj(w_sb, x_sb, dstT, po, tag, half):
                pp = slot(tag, f"gproj_{po}_{half}")
                for fo in range(FO):
                    for c in range(2):
                        cl = slice(half * ISUP + c * 512,
                                   half * ISUP + (c + 1) * 512)
                        nc.tensor.matmul(
                            pp[:, c * 512:(c + 1) * 512],
                            w_sb[:, fo, po * P:(po + 1) * P],
                            x_sb[:, fo, cl],
                            start=(fo == 0),
                            stop=(fo == FO - 1),
                        )
                    yield
                nc.vector.tensor_copy(
                    dstT[:, po, half * ISUP:(half + 1) * ISUP], pp[:]
                )
                yield

            # PE warm-up: scratch matmuls bridge the input-DMA wait so the
            # p-state ramp reaches full clock before the real projections
            # (cold PE runs them at 2-4x the cycle time)
            warm = slot("st1", "warmup")
            for _ in range(60):
                nc.tensor.matmul(
                    warm[0:1, 0:DK],
                    ones64[DK:DK + 1, 0:1],
                    ones64[DK:DK + 1, :],
                    start=True, stop=True,
                )

            # upfront: K po0 + Q po0 half0; ordering puts only the K-h0
            # and Q-h0 copies ahead of the first scores on DVE
            emit_qk_proj(wk_sb, xk_sb, KT, 0, ("st0",), halves=(0,),
                         split_copy=True)
            emit_qk_proj(wq_sb, xq_sb, QT, 0, ("st0",), halves=(0,),
                         split_copy=True)
            emit_qk_proj(wk_sb, xk_sb, KT, 0, ("st1",), halves=(1,),
                         split_copy=True)
            emit_v_proj(0, "xp1")

            # ---- norm: r = 1/Z (eps negligible: Z ~ 2e3), lane-shift to
            # partition 0, broadcast, scale ----
            def emit_norm(h, su, xp, c0=0, c1=1, w=ISUP):
                for c in range(c0, c1):
                    sl = slice(su * ISUP + c * w, su * ISUP + (c + 1) * w)
                    csl = slice(c * w, (c + 1) * w)
                    zt = npool.tile([P, ISUP], f32, tag="zt")
                    zrow = zt[DK:DK + 1, csl]
                    nc.vector.reciprocal(zrow, xp[DK:DK + 1, csl])
                    zr0 = npool.tile([1, ISUP], f32, tag="zr0")
                    nc.sync.dma_start(out=zr0[:, csl], in_=zrow)
                    rb = npool.tile([DK, ISUP], f32, tag="rb")
                    nc.gpsimd.partition_broadcast(rb[:, csl], zr0[:, csl])
                    if h < 2:
                        dst = xT[(h // 2) * DK:(h // 2) * DK + DK, h % 2, sl]
                        nc.vector.tensor_mul(dst, xp[0:DK, csl], rb[:, csl])
                    else:
                        dst = xstg[:, h % 2, sl]
                        nc.vector.tensor_mul(dst, xp[0:DK, csl], rb[:, csl])
                        nc.sync.dma_start(out=xT[DK:P, h % 2, sl], in_=dst)

            # ---- out-projection chunk: contraction od=256 as 2 matmuls ----
            def emit_outproj_chunk(su, op_tile, m, c, eng):
                sl = slice(su * ISUP + c * 512, su * ISUP + (c + 1) * 512)
                for r in range(2):
                    nc.tensor.matmul(
                        op_tile[:, 0:512],
                        wo_sb[:, r, m * P:(m + 1) * P],
                        xT[:, r, sl],
                        start=(r == 0),
                        stop=(r == 1),
                    )
                osb = opool.tile([P, 512], bf16, tag="osb")
                if eng == "act":
                    nc.scalar.copy(osb[:], op_tile[:, 0:512])
                    nc.scalar.dma_start(
                        out=outT[m * P:(m + 1) * P, sl], in_=osb[:])
                else:
                    nc.vector.tensor_copy(osb[:], op_tile[:, 0:512])
                    nc.sync.dma_start(
                        out=outT[m * P:(m + 1) * P, sl], in_=osb[:])

            # ---- attention ----
            pending = [None]

            def dance_cb():
                if pending[0] is not None:
                    emit_norm(*pending[0])
                    pending[0] = None

            op_su0 = [None]
            # deferred PE work, drip-fed one granule per jt:
            #   idx0: V tg1-3 (xp1; tg0 is upfront), then K po1 h0 (xp1)
            #   idx1: K po1 h1, Q po1 h0/h1, Q po0 h1 (xp0, free after
            #         the idx0 dance)
            # consumers: idx2 needs K po1 + Q po1 h0; idx4 Q po1 h1;
            # idx6 Q po0 h1
            def dwork0():
                # xp1 is free until idx1 starts
                for tg in (1, 2, 3):
                    yield from gen_v_proj(tg, "xp1")
                yield from gen_qk_proj(wk_sb, xk_sb, KT, 1, "xp1", 0)

            def dwork1():
                # xp0 is freed by the idx0 norm at idx1-jt2
                yield from gen_qk_proj(wk_sb, xk_sb, KT, 1, "xp0", 1)
                yield from gen_qk_proj(wq_sb, xq_sb, QT, 1, "xp0", 0)

            def dwork2():
                # xp1 is freed by the idx1 norm at idx2-jt2
                yield from gen_qk_proj(wq_sb, xq_sb, QT, 1, "xp1", 1)
                yield from gen_qk_proj(wq_sb, xq_sb, QT, 0, "xp1", 1)

            dgens = {0: dwork0(), 1: dwork1(), 2: dwork2()}

            def drip(idx, n=1):
                g = dgens.get(idx)
                if g is None:
                    return
                try:
                    for _ in range(n):
                        next(g)
                except StopIteration:
                    dgens[idx] = None

            # su0 runs po0 heads first (po1 projections are computed in
            # the background during idx0/idx1); su1 leads with heads 2/3
            # so the tail-critical last pair is head 0 (cheap norm)
            pairs = [(0, 1), (0, 0), (0, 2), (0, 3),
                     (1, 2), (1, 3), (1, 1), (1, 0)]
            for idx, (su, h) in enumerate(pairs):
                isl = su * ISUP
                hb = (h % 2) * DK
                po = h // 2
                xp = slot(f"xp{idx % 2}", f"xp_{su}_{h}")

                def scores(jt):
                    st = slot(f"st{jt % 2}", f"st_{su}_{h}_{jt}")
                    for c in range(2):
                        nc.tensor.matmul(
                            st[:, c * 512:(c + 1) * 512],
                            KT[hb:hb + DK, po, jt * P:(jt + 1) * P],
                            QT[hb:hb + DK, po,
                               isl + c * 512:isl + (c + 1) * 512],
                            start=True,
                            stop=True,
                        )
                    return st

                st_prev = scores(0)
                for jt in range(JT):
                    ett = epool.tile([P, ISUP], f32r, tag="et")
                    nc.scalar.activation(
                        ett[:], st_prev[:], EXP,
                        bias=eb_sb[:, h, jt:jt + 1], scale=0.125,
                    )
                    if jt + 1 < JT:
                        st_prev = scores(jt + 1)
                    if idx == 0:
                        # V deadlines force ~2 granules/jt early on
                        drip(0, 2 if jt < 8 else 1)
                    elif idx in (1, 2) and 2 < jt:
                        # ~10 granules over jts 3..15, inside PE slack
                        drip(idx, 1 if jt % 4 != 1 else 2)
                    for c in range(2):
                        nc.tensor.matmul(
                            xp[0:DK + 1, c * 512:(c + 1) * 512],
                            V2[:, jt, h, :],
                            ett[:, c * 512:(c + 1) * 512],
                            start=(jt == 0),
                            stop=(jt == JT - 1),
                        )
                    if idx > 0 and jt == 2:
                        dance_cb()
                    if idx == NH:
                        # su0 fully normed; spread its out-projection on
                        # xp1 (idx3's slot, freed by the dance)
                        if jt == 4:
                            op_su0[0] = slot("xp1", "op_su0")
                        if 4 <= jt < 12 and jt % 2 == 0:
                            emit_outproj_chunk(
                                0, op_su0[0], (jt - 4) // 2, 0, "dve")
                        elif 4 <= jt < 12:
                            emit_outproj_chunk(
                                0, op_su0[0], (jt - 5) // 2, 1, "dve")
                pending[0] = (h, su, xp)

            # tail: last norm in halves; out-projection chunks over four
            # psum slots with ACT/DVE alternating copies
            (lh, lsu, lxp) = pending[0]
            pending[0] = None
            # tail norm, phase-grouped: recips -> PE-broadcast into a
            # PSUM slot (no DMA lane-shift / Pool launch on the critical
            # path) -> muls; then 8 chunks over a 3-slot rotation
            ops = [slot("xp0", "op_su1_a"), slot("st0", "op_su1_b"),
                   slot("xp1", "op_su1_c")]
            rb_ps = slot("st1", "op_su1_rb")
            zt = npool.tile([P, ISUP], f32r, tag="zt2")
            with nc.allow_low_precision(reason="1/Z fits f32r"):
                for c in range(2):
                    csl = slice(c * 512, (c + 1) * 512)
                    nc.vector.reciprocal(zt[DK:DK + 1, csl],
                                         lxp[DK:DK + 1, csl])
            for c in range(2):
                csl = slice(c * 512, (c + 1) * 512)
                nc.tensor.matmul(
                    rb_ps[0:DK, csl],
                    ones64[DK:DK + 1, :],
                    zt[DK:DK + 1, csl],
                    start=True, stop=True,
                )
            rb_sb = npool.tile([DK, ISUP], f32, tag="rbsb")
            for c in range(2):
                sl = slice(lsu * ISUP + c * 512, lsu * ISUP + (c + 1) * 512)
                csl = slice(c * 512, (c + 1) * 512)
                nc.vector.tensor_copy(rb_sb[:, csl], rb_ps[0:DK, csl])
                nc.vector.tensor_mul(
                    xT[(lh // 2) * DK:(lh // 2) * DK + DK, lh % 2, sl],
                    lxp[0:DK, csl],
                    rb_sb[:, csl],
                )
            for i, (m, c) in enumerate(
                [(0, 0), (1, 0), (2, 0), (3, 0),
                 (0, 1), (1, 1), (2, 1), (3, 1)]
            ):
                emit_outproj_chunk(1, ops[i % 3], m, c,
                                   "dve" if i in (2, 5) else "act")

    nc.compile()
    return nc


def _prep_in_maps(query, key, value, mask, Wq, bq, Wk, bk, Wv, bv, Wo):
    f16np = np.float16
    B = query.shape[0]

    kfull = [key[b] @ Wk.T for b in range(B)]  # [T, 512] (no bk; see header)
    mrow = [np.where(mask[b, 0, :] == 0, np.float32(NEG), np.float32(0.0))
            for b in range(B)]

    def packT(x, npdt):
        # [T, F] -> [P, FO, T]
        return np.ascontiguousarray(
            x.T.reshape(FO, P, T).transpose(1, 0, 2)
        ).astype(npdt)

    def packW(w):
        # w [OB, F] -> [P, FO, OB] fp16
        return np.ascontiguousarray(
            w.T.reshape(FO, P, OB).transpose(1, 0, 2)
        ).astype(f16np)

    # woT[p, r, f] = Wo[f, ob0 + od(p, r)], od = ((p//64)*2 + r)*64 + p%64
    pp, rr = np.meshgrid(np.arange(P), np.arange(2), indexing="ij")
    od_map = ((pp // DK) * 2 + rr) * DK + (pp % DK)  # [P, 2]

    in_maps = []
    for c in range(8):
        b = c // 2
        hh = c % 2
        ob = slice(hh * OB, (hh + 1) * OB)
        kb = kfull[b][:, ob]
        bqb = bq[ob]
        eb = np.einsum(
            "jhd,hd->jh",
            kb.reshape(T, NH, DK),
            bqb.reshape(NH, DK),
        ) / 8.0
        eb = eb + mrow[b][:, None]
        eb = np.ascontiguousarray(
            eb.reshape(JT, P, NH).transpose(1, 2, 0)
        ).astype(np.float32)

        wo_blk = Wo[:, ob]
        woTv = np.ascontiguousarray(wo_blk.T[od_map, :]).astype(np.float32)

        in_maps.append({
            "xqT": packT(query[b], f16np),
            "xkT": packT(key[b], f16np),
            "xvT": packT(value[b], f16np),
            "wqT": packW(Wq[ob, :]),
            "wkT": packW(Wk[ob, :]),
            "wvT": packW(Wv[ob, :]),
            "woT": woTv,
            "ebias": eb,
        })
    return in_maps


def kernel(query, key, value, mask, Wq, bq, Wk, bk, Wv, bv, Wo, bo):
    query = np.asarray(query, dtype=np.float32)
    key = np.asarray(key, dtype=np.float32)
    value = np.asarray(value, dtype=np.float32)
    mask = np.asarray(mask)
    Wq = np.asarray(Wq, dtype=np.float32)
    bq = np.asarray(bq, dtype=np.float32)
    Wk = np.asarray(Wk, dtype=np.float32)
    bk = np.asarray(bk, dtype=np.float32)
    Wv = np.asarray(Wv, dtype=np.float32)
    bv = np.asarray(bv, dtype=np.float32)
    Wo = np.asarray(Wo, dtype=np.float32)
    bo = np.asarray(bo, dtype=np.float32)

    if "nc" not in _CACHE:
        _CACHE["nc"] = _build()
    nc = _CACHE["nc"]

    B = query.shape[0]
    in_maps = _prep_in_maps(
        query, key, value, mask, Wq, bq, Wk, bk, Wv, bv, Wo
    )
    res = run_bass_kernel_spmd(nc, in_maps, core_ids=list(range(8)))

    obias = bo + Wo @ bv
    out = np.empty((B, T, F), dtype=np.float32)
    for b in range(B):
        acc = (res.results[2 * b]["outT"].astype(np.float32)
               + res.results[2 * b + 1]["outT"].astype(np.float32))
        out[b] = acc.T + obias[None, :]
    return out
